# revision 56
# baseline (speedup 1.0000x reference)
# DeepseekV3MoECalibrate Trainium2 kernel (8 NeuronCores, expert-parallel).
#
# Sharding: 32 experts -> 4 per core; shared expert split along the 2I=2048
# intermediate dim (256 rows per core); tokens replicated; partial outputs
# summed with an on-device ReduceScatter.
#
# All weights and the token matrix are pre-transposed AND pre-packed on the
# HOST into the exact [128, free] SBUF layouts the PE needs, so the
# TensorEngine runs nothing but full-rate matmuls (no on-device transposes,
# no weight PSUM-evacuation copies) and every weight matrix is a single
# large DMA (per-DMA queue overhead ~0.9us makes small transfers expensive).
#
# Stage-1 (gate/up projections) runs in fp8e4 DoubleRow perf mode (K=256
# per instruction, 0.5 cycles/row) using a hi/lo fp8 split of both operands:
#   W.X ~= Wh.Xh + Wl.Xh + Wh.Xl   (error ~0.2-0.4%, vs the 2e-2 gate)
# Operands are pre-scaled by powers of two on the host (x*4, w*256) to
# avoid the fp8 denormal range; the 1/1024 descale is applied exactly via
# the silu's input scale and folded out of the up-path at the stage-3
# PSUM evacuation (tensor_scalar instead of tensor_copy, same cost).
# Stage-3 (down projection) stays fp16: one 18-matmul PSUM accumulation
# chain per output tile (4 experts x 4 i-tiles + 2 shared i-tiles).
#
# Router logits are computed exactly from an fp16 hi/lo split of x and
# gate_w (x.gw = xh.gh + xh.gl + xl.gh, error ~1e-7), so top-k selection
# matches the fp32 reference; the rest of the router is fp32 on DVE/Act.
# PSUM start_tensor_calc marks the whole 2KB zero region pending-zero, so
# only the very first matmul into the shared logits tile sets start=True.
#
# Routing weights are applied to the stage-1 activations with a deferred
# in-place scale pass on the Pool engine.
from contextlib import ExitStack

import numpy as np

import concourse.bass as bass
import concourse.tile as tile
from concourse import bacc, mybir
from concourse.masks import make_identity

F32 = mybir.dt.float32
F32R = mybir.dt.float32r
F16 = mybir.dt.float16
F8 = mybir.dt.float8e4
PM = mybir.MatmulPerfMode
AF = mybir.ActivationFunctionType
OP = mybir.AluOpType
AX = mybir.AxisListType

N_CORES = 8
T, H, I, E = 1024, 1024, 512, 32
E_LOC = E // N_CORES          # 4 experts per core
ISH = 2 * I // N_CORES        # 256 shared-intermediate rows per core
TT = T // 128                 # 8 token tiles
HK = H // 128                 # 8 h k-tiles
HP = HK // 2                  # 4 h k-tile PAIRS (fp8 DoubleRow, K=256)
IK = I // 128                 # 4 i-tiles per expert
SK = ISH // 128               # 2 shared i-tiles
TH = T // 512                 # 2 t halves (stage-1 rhs width)
NH = H // 512                 # 2 h halves (stage-3 rhs width)

SX = 4.0                      # fp8 scale on x
SW = 256.0                    # fp8 scale on gate/up weights
CINV = 1.0 / (SX * SW)        # descale folded into silu-scale / evacuation

# entry table: (kind, expert idx or None, #i-tiles); shared first so phase A
# can start before the router finishes (no routing weight needed).
ENTRIES = [("shared", None, SK)] + [("expert", e, IK) for e in range(E_LOC)]
N_ITILES = SK + E_LOC * IK    # 18 i-tiles total


def build_module(use_collective=True, num_devices=N_CORES):
    nc = bacc.Bacc("TRN2", target_bir_lowering=False, debug=False,
                   num_devices=num_devices)

    # router operands (fp16 exact-split path)
    xh_d = nc.dram_tensor("xh", [H, T], F16, kind="ExternalInput")
    xl_d = nc.dram_tensor("xl", [H, T], F16, kind="ExternalInput")
    ghl_d = nc.dram_tensor("ghl", [128, HK * 2 * E], F16, kind="ExternalInput")
    bias_d = nc.dram_tensor("bias", [128, E], F32, kind="ExternalInput")
    wselbc_d = nc.dram_tensor("wselbc", [E, E_LOC * 128], F32,
                              kind="ExternalInput")
    # stage-1 fp8 DoubleRow operands: x packed [hp][128, (j, T)] hi/lo,
    # gate/up packed [128, (s=hi/lo, hp, j, I)]
    x8h_d = nc.dram_tensor("x8h", [HP, 128, 2 * T], F8, kind="ExternalInput")
    x8l_d = nc.dram_tensor("x8l", [HP, 128, 2 * T], F8, kind="ExternalInput")
    wg_d = nc.dram_tensor("wg", [E_LOC, 128, 2 * HP * 2 * I], F8,
                          kind="ExternalInput")
    wu_d = nc.dram_tensor("wu", [E_LOC, 128, 2 * HP * 2 * I], F8,
                          kind="ExternalInput")
    sg_d = nc.dram_tensor("sg", [128, 2 * HP * 2 * ISH], F8,
                          kind="ExternalInput")
    su_d = nc.dram_tensor("su", [128, 2 * HP * 2 * ISH], F8,
                          kind="ExternalInput")
    # stage-3 expert down weights fp8 hi/lo pair-packed [128,(s,ip,j,H)];
    # shared down stays fp16 (one i-tile cannot form a DoubleRow pair)
    wd_d = nc.dram_tensor("wd", [E_LOC, 128, 2 * 2 * 2 * H], F8,
                          kind="ExternalInput")
    sd_d = nc.dram_tensor("sd", [128, 2 * 2 * H], F8,
                      kind="ExternalInput")
    out_rows = T // num_devices if use_collective else T
    out_d = nc.dram_tensor("out", [out_rows, H], F32, kind="ExternalOutput")

    with tile.TileContext(nc) as tc, ExitStack() as ctx:
        const = ctx.enter_context(tc.tile_pool(name="const", bufs=1))
        sbr = ctx.enter_context(tc.tile_pool(name="router", bufs=2))
        xpool = ctx.enter_context(tc.tile_pool(name="xt", bufs=1))
        x8pool = ctx.enter_context(tc.tile_pool(name="x8", bufs=1))
        xlp = ctx.enter_context(tc.tile_pool(name="xl", bufs=1))
        wgu_pool = ctx.enter_context(tc.tile_pool(name="wgu", bufs=1))
        wd_pool = ctx.enter_context(tc.tile_pool(name="wd", bufs=1))
        a_pool = ctx.enter_context(tc.tile_pool(name="ats", bufs=1))
        wb_pool = ctx.enter_context(tc.tile_pool(name="wb", bufs=1))
        tmp_pool = ctx.enter_context(tc.tile_pool(name="tmp", bufs=3))
        stg_pool = ctx.enter_context(tc.tile_pool(name="stg", bufs=2))
        dram = ctx.enter_context(tc.tile_pool(name="dram", bufs=1, space="DRAM"))

        ps_main = ctx.enter_context(tc.tile_pool(name="ps_main", bufs=5,
                                                 space="PSUM"))
        ps_r = ctx.enter_context(tc.tile_pool(name="ps_r", bufs=2,
                                              space="PSUM"))
        ps_lg = ctx.enter_context(tc.tile_pool(name="ps_lg", bufs=1,
                                               space="PSUM"))

        ident_f = const.tile([128, 128], F32, name="ident_f")

        # ---- DMA plan ------------------------------------------------------
        # One serial DMA stream (~344 GB/s): shared fp8 weights, x fp8 pairs
        # (pace the first chains), e0 weights interleaved with the router's
        # fp16 x tiles, xl stream, router smalls, e1..e3, down weights, outs.
        sg_sb = wgu_pool.tile([128, 2 * HP * 2 * ISH], F8, name="sg_sb")
        nc.sync.dma_start(sg_sb[:], sg_d[:])
        x8h = [x8pool.tile([128, 2 * T], F8, name=f"x8h{hp}")
               for hp in range(HP)]
        x8l = [x8pool.tile([128, 2 * T], F8, name=f"x8l{hp}")
               for hp in range(HP)]
        nc.sync.dma_start(x8h[0][:], x8h_d[0])
        nc.sync.dma_start(x8l[0][:], x8l_d[0])
        su_sb = wgu_pool.tile([128, 2 * HP * 2 * ISH], F8, name="su_sb")
        nc.sync.dma_start(su_sb[:], su_d[:])
        for hp in range(1, HP):
            nc.sync.dma_start(x8h[hp][:], x8h_d[hp])
            nc.sync.dma_start(x8l[hp][:], x8l_d[hp])

        wg_sb, wu_sb = [sg_sb], [su_sb]
        for e in range(E_LOC):
            g = wgu_pool.tile([128, 2 * HP * 2 * I], F8, name=f"wg{e}",
                              tag="wg", bufs=2)
            u = wgu_pool.tile([128, 2 * HP * 2 * I], F8, name=f"wu{e}",
                              tag="wu", bufs=2)
            wg_sb.append(g)
            wu_sb.append(u)
        nc.sync.dma_start(wg_sb[1][:], wg_d[0])
        ghl_sb = sbr.tile([128, HK * 2 * E], F16, name="ghl_sb")
        nc.sync.dma_start(ghl_sb[:], ghl_d[:])
        xt = [xpool.tile([128, T], F16, name=f"xt{ht}", tag="xt",
                         bufs=5) for ht in range(HK)]
        for ht in range(4):
            nc.sync.dma_start(xt[ht][:], xh_d[ht * 128:(ht + 1) * 128, :])
        nc.sync.dma_start(wu_sb[1][:], wu_d[0])
        for ht in range(4, HK):
            nc.sync.dma_start(xt[ht][:], xh_d[ht * 128:(ht + 1) * 128, :])

        wd_sb = [wd_pool.tile([128, 2 * 2 * H], F8, name="sd_sb")]
        for e in range(E_LOC):
            wd_sb.append(wd_pool.tile([128, 2 * 2 * 2 * H], F8,
                          name=f"wd{e}"))

        def late_dmas():
            # issued after the xl stream in queue order
            for e in range(1, E_LOC):
                nc.sync.dma_start(wg_sb[1 + e][:], wg_d[e])
                nc.sync.dma_start(wu_sb[1 + e][:], wu_d[e])
            nc.sync.dma_start(wd_sb[0][:], sd_d[:])
            for e in range(E_LOC):
                nc.sync.dma_start(wd_sb[1 + e][:], wd_d[e])

        make_identity(nc, ident_f[:])
        bias_bc = sbr.tile([128, E], F32, name="bias_bc")
        wselbc_sb = sbr.tile([E, E_LOC * 128], F32R, name="wselbc_sb")

        a_tiles = [a_pool.tile([128, T], F16, name=f"a{i}", tag="ast",
                               bufs=16) for i in range(N_ITILES)]
        a8h = {(e, ip): a_pool.tile([128, 2 * T], F8, name=f"a8h{e}_{ip}")
               for e in range(E_LOC) for ip in range(2)}
        a8l = {(e, ip): a_pool.tile([128, 2 * T], F8, name=f"a8l{e}_{ip}")
               for e in range(E_LOC) for ip in range(2)}
        a8h_sh = a_pool.tile([128, 2 * T], F8, name="a8h_sh")
        a8l_sh = a_pool.tile([128, 2 * T], F8, name="a8l_sh")
        for t8 in (a8h_sh, a8l_sh):
            nc.gpsimd.memset(
                t8[:].rearrange("p (j t) -> p j t", j=2)[:, 1, :], 0.0)

        def a8_sl(t8, j, th):
            return t8[:].rearrange("p (j t) -> p j t", j=2)[
                :, j, th * 512:(th + 1) * 512]
        a_base = {}
        off = 0
        for ei, (kind, e, ike) in enumerate(ENTRIES):
            a_base[ei] = off
            off += ike

        # ---- router: exact fp16-split logits -------------------------------
        lgall = ps_lg.tile([128, TT * E], F32, name="lgall")

        def gh_sl(ht):
            return ghl_sb[:, ht * 2 * E:ht * 2 * E + E]

        def gl_sl(ht):
            return ghl_sb[:, ht * 2 * E + E:(ht + 1) * 2 * E]

        def logits12_group(ht):
            # xh.gh + xh.gl terms (no xl dependency).  Only the very first
            # matmul into lgall's zero region may set start=True.
            for pi, rh in enumerate((gh_sl(ht), gl_sl(ht))):
                for tt in range(TT):
                    nc.tensor.matmul(
                        lgall[:, tt * E:(tt + 1) * E],
                        xt[ht][:, tt * 128:(tt + 1) * 128],
                        rh,
                        start=(ht == 0 and pi == 0 and tt == 0), stop=False,
                        skip_group_check=True)

        def logits3_group(ht):
            # xl.gh correction term
            xlt = xlp.tile([128, T], F16, name=f"xl{ht}", tag="xl", bufs=3)
            nc.sync.dma_start(xlt[:], xl_d[ht * 128:(ht + 1) * 128, :])
            for tt in range(TT):
                nc.tensor.matmul(
                    lgall[:, tt * E:(tt + 1) * E],
                    xlt[:, tt * 128:(tt + 1) * 128],
                    gh_sl(ht),
                    start=False, stop=(ht == HK - 1),
                    skip_group_check=True)

        # ---- stage 1: fp8 DoubleRow gate/up chains -------------------------
        # 12 matmuls per PSUM: (Wh.Xh, Wl.Xh, Wh.Xl) per h-pair hp=0..3.
        def w_sl(wt_, s, hp, it):
            # [p, (s, hp, j, i)] -> [p, 2, 128] slice for (s, hp, i-tile)
            v = wt_[:].rearrange("p (s hp j i) -> p s hp j i", s=2, hp=HP, j=2)
            return v[:, s, hp, :, it * 128:(it + 1) * 128]

        def x_sl(xt8, th):
            return xt8[:].rearrange("p (j t) -> p j t", j=2)[
                :, :, th * 512:(th + 1) * 512]

        def s1_chain(psum, wt_, it, th, ike):
            n = 0
            for hp in range(HP):
                # (Wh.Xh), (Wl.Xh), (Wh.Xl)
                for sw, xs in ((0, x8h[hp]), (1, x8h[hp]), (0, x8l[hp])):
                    nc.tensor.matmul(
                        psum[:], w_sl(wt_, sw, hp, it), x_sl(xs, th),
                        start=(n == 0), stop=(n == 3 * HP - 1),
                        perf_mode=PM.DoubleRow)
                    n += 1

        def stage2(ei, ab, it, th, gp, up):
            sg_t = tmp_pool.tile([128, 512], F32, name=f"sl{ei}_{th}_{it}",
                                 tag="silu")
            nc.scalar.activation(sg_t[:], gp[:], AF.Silu, scale=CINV)
            if ei == 0:
                sl = a_tiles[0][:, th * 512:(th + 1) * 512]
                nc.vector.tensor_tensor(sl, sg_t[:], up[:], OP.mult)
                nc.vector.tensor_scalar(sl, sl, 1.0 / 32.0, None, OP.mult)
                nc.gpsimd.tensor_copy(a8_sl(a8h_sh, 0, th), sl)
                nc.gpsimd.tensor_tensor(a8_sl(a8l_sh, 0, th), sl,
                                        a8_sl(a8h_sh, 0, th), OP.subtract)
                return
            nc.vector.tensor_tensor(
                ab[it][:, th * 512:(th + 1) * 512], sg_t[:], up[:], OP.mult)

        # shared entry, th=0: hp-outer across all four PSUM chains so the PE
        # keeps pace with the arriving x8 pairs.
        def stage_a0(interleave_it):
            ike = SK
            ab = a_tiles[0:SK]
            gps = [ps_main.tile([128, 512], F32, name=f"gp0_0_{it}", tag="ps")
                   for it in range(ike)]
            ups = [ps_main.tile([128, 512], F32, name=f"up0_0_{it}", tag="ps")
                   for it in range(ike)]
            for hp in range(HP):
                terms = ((0, x8h[hp], 0), (1, x8h[hp], 1), (2, x8l[hp], 0))
                for it in range(ike):
                    for n3, xs, sw in terms:
                        nc.tensor.matmul(
                            gps[it][:], w_sl(sg_sb, sw, hp, it), x_sl(xs, 0),
                            start=(hp == 0 and n3 == 0),
                            stop=(hp == HP - 1 and n3 == 2),
                            perf_mode=PM.DoubleRow)
                    for n3, xs, sw in terms:
                        nc.tensor.matmul(
                            ups[it][:], w_sl(su_sb, sw, hp, it), x_sl(xs, 0),
                            start=(hp == 0 and n3 == 0),
                            stop=(hp == HP - 1 and n3 == 2),
                            perf_mode=PM.DoubleRow)
            for it in range(ike):
                stage2(0, ab, it, 0, gps[it], ups[it])
            for it in range(ike):
                gp = ps_main.tile([128, 512], F32, name=f"gp0_1_{it}",
                                  tag="ps")
                up = ps_main.tile([128, 512], F32, name=f"up0_1_{it}",
                                  tag="ps")
                s1_chain(gp, sg_sb, it, 1, ike)
                s1_chain(up, su_sb, it, 1, ike)
                stage2(0, ab, it, 1, gp, up)
                interleave_it(it)

        def stage_a(ei, interleave=None, gu_split_th0=False):
            kind, e, ike = ENTRIES[ei]
            wgt, wut = wg_sb[ei], wu_sb[ei]
            ab = a_tiles[a_base[ei]:a_base[ei] + ike]
            step = 0
            for th in range(TH):
                if gu_split_th0 and th == 0:
                    gps = []
                    for it in range(ike):
                        gp = ps_main.tile([128, 512], F32,
                                          name=f"gp{ei}_0_{it}", tag="ps")
                        s1_chain(gp, wgt, it, 0, ike)
                        gps.append(gp)
                        if interleave is not None:
                            interleave(step)
                        step += 1
                    for it in range(ike):
                        up = ps_main.tile([128, 512], F32,
                                          name=f"up{ei}_0_{it}", tag="ps")
                        s1_chain(up, wut, it, 0, ike)
                        stage2(ei, ab, it, 0, gps[it], up)
                        if interleave is not None:
                            interleave(step)
                        step += 1
                    continue
                for it in range(ike):
                    gp = ps_main.tile([128, 512], F32,
                                      name=f"gp{ei}_{th}_{it}", tag="ps")
                    up = ps_main.tile([128, 512], F32,
                                      name=f"up{ei}_{th}_{it}", tag="ps")
                    s1_chain(gp, wgt, it, th, ike)
                    s1_chain(up, wut, it, th, ike)
                    stage2(ei, ab, it, th, gp, up)
                    if interleave is not None:
                        interleave(step)
                    step += 1

        # ---- router top-k math (DVE/Act only; transposes deferred) --------
        wt_tiles = []

        def routing_math(tt):
            lg = lgall[:, tt * E:(tt + 1) * E]
            S = sbr.tile([128, E], F32, name=f"S{tt}", tag="S")
            nc.scalar.activation(S[:], lg, AF.Sigmoid)
            SC = sbr.tile([128, E], F32, name=f"SC{tt}", tag="SC")
            nc.vector.tensor_tensor(SC[:], S[:], bias_bc[:], OP.add)
            topg = sbr.tile([128, E], F32, name=f"topg{tt}", tag="topg")
            for g in range(4):
                nc.vector.max(topg[:, 8 * g:8 * g + 8], SC[:, 8 * g:8 * g + 8])
            gs8 = sbr.tile([128, 8], F32, name=f"gs8{tt}", tag="gs8")
            nc.vector.memset(gs8[:], -1e30)
            tg = topg[:].rearrange("p (g k) -> p g k", k=8)
            nc.vector.tensor_tensor(gs8[:, 0:4], tg[:, :, 0], tg[:, :, 1],
                                    OP.add)
            gtop = sbr.tile([128, 8], F32, name=f"gtop{tt}", tag="gtop")
            nc.vector.max(gtop[:], gs8[:])
            gmask = sbr.tile([128, 4], F32, name=f"gmask{tt}", tag="gmask")
            nc.vector.tensor_scalar(gmask[:], gs8[:, 0:4], gtop[:, 1:2], None,
                                    OP.is_ge)
            SCm = sbr.tile([128, E], F32, name=f"SCm{tt}", tag="SCm")
            nc.vector.tensor_tensor(
                SCm[:].rearrange("p (g k) -> p g k", k=8),
                SC[:].rearrange("p (g k) -> p g k", k=8),
                gmask[:].rearrange("p (g k) -> p g k", k=1).broadcast_to(
                    [128, 4, 8]),
                OP.mult)
            etop = sbr.tile([128, 8], F32, name=f"etop{tt}", tag="etop")
            nc.vector.max(etop[:], SCm[:])
            sel = sbr.tile([128, E], F32, name=f"sel{tt}", tag="sel")
            nc.vector.tensor_scalar(sel[:], SCm[:], etop[:, 7:8], None,
                                    OP.is_ge)
            wr = sbr.tile([128, E], F32, name=f"wr{tt}", tag="wr")
            nc.vector.tensor_tensor(wr[:], S[:], sel[:], OP.mult)
            den = sbr.tile([128, 1], F32, name=f"den{tt}", tag="den")
            nc.vector.reduce_sum(den[:], wr[:], axis=AX.X)
            # the x2.5 routed scaling is folded into wselbc on the host
            dinv = sbr.tile([128, 1], F32, name=f"dinv{tt}", tag="dinv")
            nc.vector.reciprocal(dinv[:], den[:])
            wt = sbr.tile([128, E], F32, name=f"wt{tt}", tag="wt", bufs=8)
            nc.vector.tensor_scalar(wt[:], wr[:], dinv[:], None, OP.mult)
            wt_tiles.append(wt)

        # ================= emission schedule ===============================
        # Shared entry first (needs no routing weights); logits groups with
        # no xl dependency slot into its th=1 steps and expert-0's first
        # steps, the xl correction term into expert-0's later steps.
        stage_a0(interleave_it=lambda it: None)

        def e0_hook(s):
            if s < 8:
                logits12_group(s)
            elif s < 12:
                logits3_group(2 * (s - 8))
                logits3_group(2 * (s - 8) + 1)
            if s == 11:
                # routing math right behind the last logits write so the
                # static scheduler orders it ahead of e1's stage-2 work
                for tt in range(TT):
                    routing_math(tt)

        nc.sync.dma_start(bias_bc[:], bias_d[:])
        nc.sync.dma_start(wselbc_sb[:], wselbc_d[:].bitcast(F32R))
        stage_a(1, interleave=e0_hook, gu_split_th0=True)
        late_dmas()

        stage_a(2)

        # wt transposes + routing-weight broadcast rows; placed after e1's
        # stage-1 so the PE arrives here well after the DVE router finishes.
        wT_r = sbr.tile([E, T], F32R, name="wT_r")
        for tt in range(TT):
            p = ps_r.tile([128, 512], F32, name=f"wtp{tt}", tag="ps_r")
            nc.tensor.transpose(p[0:E, 0:128], wt_tiles[tt][:], ident_f[:])
            nc.vector.tensor_copy(wT_r[:, tt * 128:(tt + 1) * 128].bitcast(F32R),
                                  p[0:E, 0:128].bitcast(F32R))
        wb_tiles = []
        for e in range(E_LOC):
            wbt = wb_pool.tile([128, T], F16, name=f"wb{e}")
            for th in range(TH):
                p = ps_r.tile([128, 512], F32, name=f"wbp{e}_{th}", tag="ps_r")
                nc.tensor.matmul(p[:], wselbc_sb[:, e * 128:(e + 1) * 128],
                                 wT_r[:, th * 512:(th + 1) * 512],
                                 start=True, stop=True)
                nc.vector.tensor_copy(wbt[:, th * 512:(th + 1) * 512], p[:])
            wb_tiles.append(wbt)

        # deferred routing-weight scale + fp8 hi/lo split, in place; the
        # wb rows carry the 2.5/32 factor so the result is DR-pair ready.
        def convert_pass(ei, th):
            kind, e, ike = ENTRIES[ei]
            ab = a_tiles[a_base[ei]:a_base[ei] + ike]
            for it in range(ike):
                eng = nc.gpsimd if (it + th) % 2 == 0 else nc.vector
                ip, j = it // 2, it % 2
                sl = ab[it][:, th * 512:(th + 1) * 512]
                eng.tensor_tensor(
                    sl, sl, wb_tiles[e][:, th * 512:(th + 1) * 512],
                    OP.mult)
                eng.tensor_copy(a8_sl(a8h[(e, ip)], j, th), sl)
                eng.tensor_tensor(a8_sl(a8l[(e, ip)], j, th), sl,
                                  a8_sl(a8h[(e, ip)], j, th), OP.subtract)

        def scale_pass(ei):
            convert_pass(ei, 0)
            convert_pass(ei, 1)

        # all stage-1 first, THEN the wb-gated converts (th0 before th1, so
        # phase B's first chains unblock earliest) — otherwise e2/e3 stage-2
        # work queues behind converts that wait on wb.
        stage_a(3)
        stage_a(4)
        for th in range(TH):
            for ei in range(len(ENTRIES) - 1, 0, -1):
                convert_pass(ei, th)

        # ---- stage 3: one 18-matmul fp16 PSUM chain per output tile -------
        if use_collective:
            bin_t = dram.tile([T, H], F32, name="rsin")
            target = bin_t
        else:
            target = out_d
        def wd8_sl(e, s, ip, hh, c0, cw):
            return wd_sb[1 + e][:].rearrange(
                "p (s ip j h) -> p s ip j h", s=2, ip=2, j=2)[
                :, s, ip, :, hh * 512 + c0:hh * 512 + c0 + cw]

        def b_chain(tt, hh, c0, cw, tag):
            op = ps_main.tile([128, 512], F32, name=f"o{tt}_{hh}_{c0}",
                              tag="ps")
            def sd8_sl(s):
                return wd_sb[0][:].rearrange("p (s j h) -> p s j h",
                                             s=2, j=2)[
                    :, s, :, hh * 512 + c0:hh * 512 + c0 + cw]

            n = 0
            n_tot = 3 + E_LOC * 2 * 3
            for t8, s in ((a8h_sh, 0), (a8l_sh, 0), (a8h_sh, 1)):
                nc.tensor.matmul(
                    op[:, 0:cw],
                    t8[:].rearrange("p (j t) -> p j t", j=2)[
                        :, :, tt * 128:(tt + 1) * 128],
                    sd8_sl(s),
                    start=(n == 0), stop=False, perf_mode=PM.DoubleRow)
                n += 1
            for e in range(E_LOC):
                for ip in range(2):
                    for t8, s in ((a8h[(e, ip)], 0), (a8l[(e, ip)], 0),
                                  (a8h[(e, ip)], 1)):
                        nc.tensor.matmul(
                            op[:, 0:cw],
                            t8[:].rearrange("p (j t) -> p j t", j=2)[
                                :, :, tt * 128:(tt + 1) * 128],
                            wd8_sl(e, s, ip, hh, c0, cw),
                            start=False, stop=(n == n_tot - 1),
                            perf_mode=PM.DoubleRow)
                        n += 1
            st = stg_pool.tile([128, 512], F32, name=f"st{tt}_{hh}_{c0}",
                               tag=tag)
            nc.vector.tensor_scalar(st[:, 0:cw], op[:, 0:cw], CINV / 8.0,
                                    None, OP.mult)
            nc.sync.dma_start(
                target[tt * 128:(tt + 1) * 128,
                       hh * 512 + c0:hh * 512 + c0 + cw],
                st[:, 0:cw])

        for tt in range(TT):
            for hh in range(NH):
                if tt == TT - 1 and hh == NH - 1:
                    # split the final tile so its evacuation+DMA pipeline
                    # overlaps the second half instead of the drain window
                    b_chain(tt, hh, 0, 256, "stg")
                    b_chain(tt, hh, 256, 256, "stg")
                else:
                    b_chain(tt, hh, 0, 512, "stg")

        # ---- ReduceScatter + output ---------------------------------------
        if use_collective:
            bout_t = dram.tile([out_rows, H], F32, name="rsout")
            nc.gpsimd.collective_compute(
                "ReduceScatter", OP.add,
                replica_groups=[list(range(num_devices))],
                ins=[bin_t.opt()], outs=[bout_t.opt()])
            nc.sync.dma_start(out_d[:], bout_t[:])
    nc.compile()
    return nc


_NC_CACHE = {}


def _get_module():
    key = "spmd"
    if key not in _NC_CACHE:
        _NC_CACHE[key] = build_module(use_collective=True, num_devices=N_CORES)
    return _NC_CACHE[key]


def _pack_rows(a, blk=128):
    """[R, C] -> [128, (R//128) * C]: row-tile r128 layout for one-DMA loads."""
    r, c = a.shape
    return np.ascontiguousarray(
        a.reshape(r // blk, blk, c).transpose(1, 0, 2).reshape(blk, -1))


def _fp8_split(a):
    """fp8e4 hi/lo split: a ~= hi + lo (both float8_e4m3)."""
    import ml_dtypes
    hi = a.astype(ml_dtypes.float8_e4m3)
    lo = (a - hi.astype(np.float32)).astype(ml_dtypes.float8_e4m3)
    return hi, lo


def _pack_w8(wT):
    """[H, C] f32 (pre-transposed weight) -> [128, (s, hp, j, C)] fp8 pair."""
    h, c = wT.shape
    hi, lo = _fp8_split(wT * SW)
    arr = np.stack([np.asarray(hi), np.asarray(lo)])        # [2, H, C]
    arr = arr.reshape(2, HP, 2, 128, c).transpose(3, 0, 1, 2, 4)
    return np.ascontiguousarray(arr.reshape(128, 2 * HP * 2 * c))


def _pack_sd8z(sdT):
    """[128, H] f32 -> [128, (s, j, H)] fp8, j=1 rows zero (half pair)."""
    hi, lo = _fp8_split(sdT * SW)
    h = sdT.shape[1]
    arr = np.zeros((128, 2, 2, h), np.float32)
    arr[:, 0, 0, :] = np.asarray(hi).astype(np.float32)
    arr[:, 1, 0, :] = np.asarray(lo).astype(np.float32)
    import ml_dtypes
    return np.ascontiguousarray(
        arr.reshape(128, -1).astype(ml_dtypes.float8_e4m3))


def _pack_wd8(edT):
    """[I, H] f32 (pre-transposed down weight) -> [128,(s,ip,j,H)] fp8."""
    hi, lo = _fp8_split(edT * SW)
    h = edT.shape[1]
    arr = np.stack([np.asarray(hi), np.asarray(lo)])     # [2, I, H]
    arr = arr.reshape(2, 2, 2, 128, h).transpose(3, 0, 1, 2, 4)
    return np.ascontiguousarray(arr.reshape(128, -1))


def _pack_x8(xT):
    """[H, T] f32 -> hi/lo [HP, 128, (j, T)] fp8 DoubleRow layout."""
    hi, lo = _fp8_split(xT * SX)
    out = []
    for a in (hi, lo):
        b = np.asarray(a).reshape(HP, 2, 128, T).transpose(0, 2, 1, 3)
        out.append(np.ascontiguousarray(b.reshape(HP, 128, 2 * T)))
    return out


def make_in_maps(hidden_states, gate_w, gate_bias, expert_gate, expert_up,
                 expert_down, shared_gate, shared_up, shared_down):
    x = np.asarray(hidden_states, np.float32).reshape(T, H)
    xt = np.ascontiguousarray(x.T)                       # [H, T]
    xh = xt.astype(np.float16)
    xl = (xt - xh.astype(np.float32)).astype(np.float16)
    x8h, x8l = _pack_x8(xt)
    gwt = np.ascontiguousarray(np.asarray(gate_w, np.float32).T)  # [H, E]
    gh = gwt.astype(np.float16)
    gl = (gwt - gh.astype(np.float32)).astype(np.float16)
    # pack gh/gl as [128, (ht, {gh,gl}, E)]
    ghl = np.concatenate(
        [gh.reshape(HK, 128, E)[:, :, None, :],
         gl.reshape(HK, 128, E)[:, :, None, :]], axis=2)  # [HK,128,2,E]
    ghl = np.ascontiguousarray(
        ghl.transpose(1, 0, 2, 3).reshape(128, HK * 2 * E))
    bias = np.broadcast_to(
        np.asarray(gate_bias, np.float32).reshape(1, E), (128, E))
    bias = np.ascontiguousarray(bias)
    eg = np.asarray(expert_gate, np.float32)
    eu = np.asarray(expert_up, np.float32)
    ed = np.asarray(expert_down, np.float32)
    sgT = np.asarray(shared_gate, np.float32).T          # [H, 2I]
    suT = np.asarray(shared_up, np.float32).T            # [H, 2I]
    sd = np.asarray(shared_down, np.float32)             # [H, 2I]
    in_maps = []
    for c in range(N_CORES):
        lo, hi = c * E_LOC, (c + 1) * E_LOC
        wselbc = np.zeros((E, E_LOC * 128), np.float32)
        for j in range(E_LOC):
            wselbc[lo + j, j * 128:(j + 1) * 128] = 2.5 / 32.0
        wg = np.stack([_pack_w8(eg[lo + j].T) for j in range(E_LOC)])
        wu = np.stack([_pack_w8(eu[lo + j].T) for j in range(E_LOC)])
        wd = np.stack([_pack_wd8(ed[lo + j].T) for j in range(E_LOC)])
        in_maps.append({
            "xh": xh, "xl": xl, "x8h": x8h, "x8l": x8l,
            "ghl": ghl, "bias": bias, "wselbc": wselbc,
            "wg": wg, "wu": wu, "wd": wd,
            "sg": _pack_w8(np.ascontiguousarray(sgT[:, c * ISH:(c + 1) * ISH])),
            "su": _pack_w8(np.ascontiguousarray(suT[:, c * ISH:(c + 1) * ISH])),
            "sd": _pack_sd8z(np.ascontiguousarray(
                sd[:, c * ISH:(c + 1) * ISH].T)),
        })
    return in_maps


def kernel(hidden_states, gate_w, gate_bias, expert_gate, expert_up,
           expert_down, shared_gate, shared_up, shared_down):
    import os
    # The axon NTFF trace hook is absent in this container; make sure the
    # PJRT execute path never tries to use it.
    os.environ.setdefault("BASS_NEVER_TRACE", "1")
    from concourse.bass_utils import run_bass_kernel_spmd
    nc = _get_module()
    in_maps = make_in_maps(hidden_states, gate_w, gate_bias, expert_gate,
                           expert_up, expert_down, shared_gate, shared_up,
                           shared_down)
    res = run_bass_kernel_spmd(nc, in_maps, core_ids=list(range(N_CORES)))
    out = np.concatenate([np.asarray(res.results[c]["out"], np.float32)
                          for c in range(N_CORES)], axis=0)
    return out.reshape(np.asarray(hidden_states).shape)
